# revision 1
# baseline (speedup 1.0000x reference)
"""Trainium2 Bass kernel for nn_Conv2DExperimental (MVN-sampled 3x3 conv).

Computation (per the nn.Module):
  L    = tril(weight_L, -1) + softplus(diag(weight_L)) * I      # [O,I,D,D], D=9
  w    = weight_loc + einsum('oiab,oib->oia', L, eps_w)         # [O,I,3,3]
  b    = bias_loc + eps_b * softplus(bias_ro)                   # [O]
  out  = conv2d(x, w, SAME, NCHW) + b

Distribution: data-parallel over the batch dim of x (32 images -> 8 cores x 4),
with the weight sampling replicated on every core (it is tiny).

Per-core kernel:
  - sampling runs on VectorE/ScalarE with O on the partition dim; the 9 sampled
    64x64 tap matrices are PE-transposed into block-diagonal [128,128] lhsT
    tiles pairing two images per matmul (partitions = (image, channel)).
  - conv runs as 9 shift-matmuls per 2-output-row PSUM tile in float32r
    (fp32 data truncated to FP22 in the PE; 1 cycle/row at N>=256).
  - ScalarE evacuates PSUM with the bias add fused; DMA engines stream
    row-strips of x in and finished strips of out back to HBM.
"""

import sys
from contextlib import ExitStack

for _p in ("/opt/trn_rl_repo",):
    if _p not in sys.path:
        sys.path.insert(0, _p)

import numpy as np

import concourse.bass as bass
import concourse.bacc as bacc
import concourse.mybir as mybir
from concourse.tile import TileContext

F32 = mybir.dt.float32
F32R = mybir.dt.float32r
AF = mybir.ActivationFunctionType

N_CORES = 8
O = 64
I = 64
KK = 3
D = KK * KK  # 9


def build_nc(nb=4, hh=224, ww=224, rstrip=28, x_bufs=3, o_bufs=2, passes=1):
    """Build the per-core Bass program.

    nb: images per core (must be even: images are processed in pairs)
    hh, ww: spatial dims; rstrip: output rows per strip (must divide hh, even)
    """
    assert nb % 2 == 0 and hh % rstrip == 0 and rstrip % 2 == 0
    wpad = ww + 2
    nstrips = hh // rstrip
    ntiles = rstrip // 2  # psum tiles (2 output rows each) per strip

    nc = bacc.Bacc("TRN2", target_bir_lowering=False, debug=False)

    x_t = nc.dram_tensor("x", [nb, I, hh, ww], F32R, kind="ExternalInput").ap()
    wl_t = nc.dram_tensor("wL", [O, I * D * D], F32, kind="ExternalInput").ap()
    wloc_t = nc.dram_tensor("wloc", [O, I * D], F32, kind="ExternalInput").ap()
    epsw_t = nc.dram_tensor("epsw", [O, I * D], F32, kind="ExternalInput").ap()
    ident_t = nc.dram_tensor("ident", [O, O], F32, kind="ExternalInput").ap()
    bias3_t = nc.dram_tensor("bias3", [3, O], F32, kind="ExternalInput").ap()
    out_t = nc.dram_tensor("out", [nb, O, hh, ww], F32, kind="ExternalOutput").ap()

    with TileContext(nc) as tc, ExitStack() as stack:
        # ---------------- weight + bias sampling (one-time prologue) --------
        cp = stack.enter_context(tc.tile_pool(name="consts", bufs=1))
        wl = cp.tile([O, I * D * D], F32, name="wl", tag="wl")
        wloc = cp.tile([O, I * D], F32, name="wloc_s", tag="wloc_s")
        epsw = cp.tile([O, I * D], F32, name="epsw_s", tag="epsw_s")
        ident = cp.tile([O, O], F32, name="ident_s", tag="ident_s")
        b3 = cp.tile([O, 3], F32, name="b3", tag="b3")
        sp = cp.tile([O, I * D], F32, name="sp", tag="sp")
        tmp = cp.tile([O, I * D], F32, name="tmp", tag="tmp")
        wsamp = cp.tile([O, I * D], F32, name="wsamp", tag="wsamp")
        bias = cp.tile([128, 1], F32, name="bias", tag="bias")
        # 9 block-diagonal lhsT tiles, stored side by side: [128, 9*128]
        wts = cp.tile([128, D * 128], F32R, name="wts", tag="wts")
        sp_b = cp.tile([O, 1], F32, name="sp_b", tag="sp_b")

        b3p = cp.tile([3, O], F32, name="b3p", tag="b3p")
        nc.sync.dma_start(wl[:], wl_t[:])
        nc.scalar.dma_start(b3p[:], bias3_t[:])
        nc.scalar.dma_start(ident[:], ident_t[:])
        nc.scalar.dma_start(wloc[:], wloc_t[:])
        nc.scalar.dma_start(epsw[:], epsw_t[:])

        # PE warm-up feed: zero tiles via GpSimd (idle queue, no input deps)
        # so the warm-up matmuls can start within ~1us of kernel entry.
        identr = cp.tile([O, O], F32R, name="identr", tag="identr")
        junk = cp.tile([O, 256], F32R, name="junk", tag="junk")
        with tc.high_priority():
            nc.gpsimd.memset(identr[:].bitcast(F32), 0.0)
            nc.gpsimd.memset(junk[:].bitcast(F32), 0.0)
        nc.gpsimd.memset(wts[:].bitcast(F32), 0.0)

        # PE warm-up: the HAM clock gate needs ~3.4us of sustained matmul
        # activity to lift the PE from 1.2 to 2.4 GHz, and re-throttles after
        # ~3.4us idle. One long accumulation group (no inter-matmul
        # semaphores) bridges the PE from kernel entry to the transposes.
        with tc.tile_pool(name="wp", bufs=1, space="PSUM") as wp:
            warm = wp.tile([O, 256], F32, name="warm")
            n_warm = 90
            for k in range(n_warm):
                nc.tensor.matmul(
                    warm[:], identr[:], junk[:],
                    start=(k == 0), stop=(k == n_warm - 1),
                )

            # bias3 arrives as [3, 64]; transpose to [64, 3] on the PE (a
            # partition-major DMA of 64x3 elements costs ~17us in descriptors)
            bp_ps = wp.tile([O, 3], F32, name="bp_ps")
            with tc.high_priority():
                nc.tensor.matmul(
                    bp_ps[:], b3p[:], ident[0:3, 0:3], start=True, stop=True
                )
                nc.vector.tensor_copy(b3[:], bp_ps[:])

        # softplus of the per-(o,i) diagonals: wl free layout is (i, d=a*9+b);
        # diagonal entries sit at d = 10*a  ->  sp layout (i, a).
        # ACT order Exp,Exp,Ln,Ln avoids activation-table reload thrash
        # (each ACT_TABLE_LOAD costs ~1.3us). softplus(x) = ln(exp(x) + 1):
        # there is no Softplus LUT in this toolchain.
        diag_view = bass.AP(
            tensor=wl[:].tensor,
            offset=wl[:].offset,
            ap=[list(p) for p in wl[:].ap[:1]] + [[D * D, I], [D + 1, D]],
        )
        sp3 = sp[:].rearrange("o (i a) -> o i a", i=I)
        with tc.high_priority():
            nc.scalar.activation(sp_b[:], b3[:, 1:2], AF.Exp)
            nc.scalar.activation(sp3, diag_view, AF.Exp)
            nc.scalar.activation(sp[:], sp[:], AF.Ln, bias=1.0)
            nc.scalar.activation(sp_b[:], sp_b[:], AF.Ln, bias=1.0)

        # bias = bias_loc + eps_b * softplus(bias_ro)
        nc.vector.tensor_mul(sp_b[:], sp_b[:], b3[:, 2:3])
        nc.vector.tensor_add(bias[0:O, :], b3[:, 0:1], sp_b[:])
        nc.scalar.dma_start(bias[O:128, :], bias[0:O, :])

        # wsamp = wloc + softplus(diag) * eps  (the b == a term of L @ eps)
        nc.vector.tensor_mul(tmp[:], sp[:], epsw[:])
        nc.vector.tensor_add(wsamp[:], wloc[:], tmp[:])

        # += strict-lower part: for each b,
        #   wsamp[o,(i,a)] += wl[o,(i,a*9+b)] * eps[o,(i,b)]  for a in b+1..8.
        # Restricting each view to a > b IS the tril(-1) mask.
        for b in range(D - 1):
            na = D - 1 - b  # taps strictly below the diagonal
            wl_b = bass.AP(
                tensor=wl[:].tensor,
                offset=wl[:].offset + (b + 1) * D + b,
                ap=[list(p) for p in wl[:].ap[:1]] + [[D * D, I], [D, na]],
            )
            eps_b = bass.AP(
                tensor=epsw[:].tensor,
                offset=epsw[:].offset + b,
                ap=[list(p) for p in epsw[:].ap[:1]] + [[D, I], [0, na]],
            )
            tmp_b = bass.AP(
                tensor=tmp[:].tensor,
                offset=tmp[:].offset + b + 1,
                ap=[list(p) for p in tmp[:].ap[:1]] + [[D, I], [1, na]],
            )
            ws_b = bass.AP(
                tensor=wsamp[:].tensor,
                offset=wsamp[:].offset + b + 1,
                ap=[list(p) for p in wsamp[:].ap[:1]] + [[D, I], [1, na]],
            )
            nc.vector.tensor_tensor(tmp_b, wl_b, eps_b, mybir.AluOpType.mult)
            nc.vector.tensor_add(ws_b, ws_b, tmp_b)

        # build the 9 block-diagonal lhsT tiles:
        #   wts[:, a*128:(a+1)*128] = [[T_a, 0], [0, T_a]],  T_a[i,o] = wsamp[o, i*9+a]
        with tc.tile_pool(name="pt", bufs=1, space="PSUM") as ptp:
            # transpose the 9 taps, packed 5 + 4 into two PSUM banks, then
            # two strided copies into the lhsT tile (disjoint column ranges:
            # start=True only on the first write of each bank)
            ptA = ptp.tile([O, 5 * O], F32, name="ptA")
            ptB = ptp.tile([O, 4 * O], F32, name="ptB")
            for a in range(D):
                w_a = bass.AP(
                    tensor=wsamp[:].tensor,
                    offset=wsamp[:].offset + a,
                    ap=[list(p) for p in wsamp[:].ap[:1]] + [[D, I]],
                )
                dst_pt = ptA if a < 5 else ptB
                c = a if a < 5 else a - 5
                nc.tensor.matmul(
                    dst_pt[:, c * O : (c + 1) * O],
                    w_a,
                    ident[:],
                    is_transpose=True,
                    start=(c == 0),
                    stop=(c == (4 if a < 5 else 3)),
                    skip_group_check=True,
                )
            for pt_t, a0, na_t in ((ptA, 0, 5), (ptB, 5, 4)):
                dst = bass.AP(
                    tensor=wts[0:O].tensor,
                    offset=wts[0:O].offset + a0 * 128,
                    ap=[list(p) for p in wts[0:O].ap[:1]] + [[128, na_t], [1, O]],
                )
                nc.vector.tensor_copy(dst, pt_t[:].rearrange("p (a o) -> p a o", o=O))
        # partition-shifted copy of the diagonal blocks: [0:64, a*128:+64] ->
        # [64:128, a*128+64:+64]. Two DMAs (taps 0-4 / 5-8) so the first
        # conv matmuls only wait on the first transpose batch, not the whole
        # sampling chain.
        wts_lo = wts[0:O]
        wts_hi = wts[O:128]
        for a0, na_t in ((0, 5), (5, 4)):
            src = bass.AP(
                tensor=wts_lo.tensor,
                offset=wts_lo.offset + a0 * 128,
                ap=[list(p) for p in wts_lo.ap[:1]] + [[128, na_t], [1, O]],
            )
            dst = bass.AP(
                tensor=wts_hi.tensor,
                offset=wts_hi.offset + a0 * 128 + O,
                ap=[list(p) for p in wts_hi.ap[:1]] + [[128, na_t], [1, O]],
            )
            nc.scalar.dma_start(dst, src)

        # ---------------- convolution ---------------------------------------
        xp = stack.enter_context(tc.tile_pool(name="xstrip", bufs=x_bufs))
        op = stack.enter_context(tc.tile_pool(name="ostrip", bufs=o_bufs))
        pp = stack.enter_context(tc.tile_pool(name="acc", bufs=8, space="PSUM"))
        for _pass in range(passes):
            for pair in range(nb // 2):
                n0 = 2 * pair
                strips = [(s * rstrip, rstrip) for s in range(nstrips)]
                if pair == nb // 2 - 1 and _pass == passes - 1 and rstrip >= 8:
                    # Taper the final strips so the kernel does not end on a
                    # full-size store DMA the PE has to wait out.
                    h_last = strips.pop()[0]
                    r = rstrip
                    while r > 4:
                        r1 = (r // 2 + 1) & ~1
                        strips.append((h_last, r1))
                        h_last += r1
                        r -= r1
                    strips.append((h_last, r))
                for h0, rout in strips:
                    xs = xp.tile([128, rstrip + 2, wpad], F32R, name="xs")
                    # zero the left/right halo columns
                    halo = bass.AP(
                        tensor=xs[:].tensor,
                        offset=xs[:].offset,
                        ap=[list(p) for p in xs[:].ap[:1]]
                        + [[wpad, rout + 2], [ww + 1, 2]],
                    )
                    nc.gpsimd.memset(halo.bitcast(F32), 0.0)
                    # load input rows [h0-1, h0+rout], clipped to the image
                    r_lo = max(h0 - 1, 0)
                    r_hi = min(h0 + rout + 1, hh)
                    dst_r0 = r_lo - (h0 - 1)
                    if h0 == 0:
                        nc.gpsimd.memset(xs[:, 0:1, :].bitcast(F32), 0.0)
                    if h0 + rout == hh:
                        nc.gpsimd.memset(
                            xs[:, rout + 1 : rout + 2, :].bitcast(F32), 0.0
                        )
                    src = x_t[n0 : n0 + 2, :, r_lo:r_hi, :].rearrange(
                        "n i h w -> (n i) h w"
                    )
                    nc.sync.dma_start(
                        xs[:, dst_r0 : dst_r0 + (r_hi - r_lo), 1 : ww + 1], src
                    )

                    os_ = op.tile([128, rout, ww], F32, name="os_")
                    for j in range(rout // 2):
                        acc = pp.tile([128, 2, ww], F32, name="acc")
                        for tap in range(D):
                            dy, dx = tap // 3 - 1, tap % 3 - 1
                            rhs = bass.AP(
                                tensor=xs[:].tensor,
                                offset=xs[:].offset
                                + (2 * j + 1 + dy) * wpad
                                + 1
                                + dx,
                                ap=[list(p) for p in xs[:].ap[:1]]
                                + [[wpad, 2], [1, ww]],
                            )
                            nc.tensor.matmul(
                                acc[:],
                                wts[:, tap * 128 : (tap + 1) * 128],
                                rhs,
                                start=(tap == 0),
                                stop=(tap == D - 1),
                            )
                        nc.scalar.activation(
                            os_[:, 2 * j : 2 * j + 2, :],
                            acc[:],
                            AF.Identity,
                            bias=bias[:, 0:1],
                        )
                    dst = out_t[n0 : n0 + 2, :, h0 : h0 + rout, :].rearrange(
                        "n i h w -> (n i) h w"
                    )
                    nc.sync.dma_start(dst, os_[:])

    nc.compile()
    return nc


_CACHED_NC = None


def _host_inputs(x_shard, weight_loc, weight_L, bias_loc, bias_ro, eps_w, eps_b):
    return {
        "x": np.ascontiguousarray(x_shard, np.float32),
        "wL": np.ascontiguousarray(weight_L.reshape(O, I * D * D), np.float32),
        "wloc": np.ascontiguousarray(weight_loc.reshape(O, I * D), np.float32),
        "epsw": np.ascontiguousarray(eps_w.reshape(O, I * D), np.float32),
        "ident": np.eye(O, dtype=np.float32),
        "bias3": np.ascontiguousarray(
            np.stack([bias_loc, bias_ro, eps_b]).astype(np.float32)
        ),
    }


def kernel(x, weight_loc, weight_L, bias_loc, bias_ro, eps_w, eps_b):
    global _CACHED_NC
    from concourse.bass_utils import run_bass_kernel_spmd

    x = np.asarray(x, np.float32)
    nb = x.shape[0] // N_CORES
    if _CACHED_NC is None:
        _CACHED_NC = build_nc(nb=nb)
    nc = _CACHED_NC

    in_maps = [
        _host_inputs(
            x[c * nb : (c + 1) * nb],
            np.asarray(weight_loc),
            np.asarray(weight_L),
            np.asarray(bias_loc),
            np.asarray(bias_ro),
            np.asarray(eps_w),
            np.asarray(eps_b),
        )
        for c in range(N_CORES)
    ]
    res = run_bass_kernel_spmd(nc, in_maps, list(range(N_CORES)))
    return np.concatenate([res.results[c]["out"] for c in range(N_CORES)], axis=0)



# revision 16
# speedup vs baseline: 1.8365x; 1.8365x over previous
"""Trainium2 Bass kernel for nn_Conv2DExperimental (MVN-sampled 3x3 conv).

Computation (per the nn.Module):
  L    = tril(weight_L, -1) + softplus(diag(weight_L)) * I      # [O,I,D,D], D=9
  w    = weight_loc + einsum('oiab,oib->oia', L, eps_w)         # [O,I,3,3]
  b    = bias_loc + eps_b * softplus(bias_ro)                   # [O]
  out  = conv2d(x, w, SAME, NCHW) + b

Distribution: data-parallel over the batch dim of x (32 images -> 8 cores x 4),
with the weight sampling replicated on every core (it is tiny).

Per-core kernel:
  - sampling: the host uploads weight_L strict-lower-masked (structural tril
    mask, raw diagonal kept in place). The device computes softplus of the
    diagonal, scatters it back into the diagonal slots, then one big
    elementwise multiply P[o,i,a,b] = L[o,i,a,b] * eps[o,i,b] and a segmented
    reduce over b produce the sampled taps in two DVE ops.
  - conv: FOUR CONCURRENT 64x64 quadrant matmul streams on the PE
    (tile_position derives from base partitions): quadrant (r, c) reads
    x from SBUF partitions r..r+63 and writes PSUM partitions c..c+63.
    With x holding image A on partitions 0-63 and image B on 64-127, psum
    tiles alternate [A;B] / [B;A] halves so all four quadrants stream
    simultaneously -> ~100% PE array utilization (vs 50% for a 128x128
    block-diagonal lhsT). Taps iterate OUTSIDE a group of 4 psum tiles so
    consecutive matmuls reuse the loaded weights where possible.
  - SAME-padding is expressed by shortening the matmul APs at the image
    edges instead of zero-halos (no memsets in the main loop).
  - x streams in as bf16, out streams back as fp16 (halves HBM traffic);
    ScalarE evacuates PSUM with the bias add fused; the image-half swap of
    odd psum tiles is undone by strided store DMAs.
"""

import sys
from contextlib import ExitStack

for _p in ("/opt/trn_rl_repo",):
    if _p not in sys.path:
        sys.path.insert(0, _p)

import numpy as np

import concourse.bass as bass
import concourse.bacc as bacc
import concourse.mybir as mybir
from concourse.tile import TileContext

F32 = mybir.dt.float32
F32R = mybir.dt.float32r
BF16 = mybir.dt.bfloat16
FP16 = mybir.dt.float16
AF = mybir.ActivationFunctionType

N_CORES = 8
O = 64
I = 64
KK = 3
D = KK * KK  # 9

# tap order: (0,0) first so the start=True matmul covers the full psum tile
TAP_ORDER = [4, 0, 1, 2, 3, 5, 6, 7, 8]


def build_nc(nb=4, hh=224, ww=224, rstrip=32, x_bufs=3, o_bufs=2):
    """Build the per-core Bass program.

    nb: images per core (must be even: images are processed in pairs)
    hh, ww: spatial dims; rstrip: output rows per strip (multiple of 8)
    """
    assert nb % 2 == 0 and hh % rstrip == 0 and rstrip % 8 == 0
    nstrips = hh // rstrip

    nc = bacc.Bacc("TRN2", target_bir_lowering=False, debug=False)

    x_t = nc.dram_tensor("x", [nb, I, hh, ww], BF16, kind="ExternalInput").ap()
    wl_t = nc.dram_tensor("wL", [O, I * D * D], BF16, kind="ExternalInput").ap()
    wloc_t = nc.dram_tensor("wloc", [O, I * D], F32, kind="ExternalInput").ap()
    epsw_t = nc.dram_tensor("epsw", [O, I * D], BF16, kind="ExternalInput").ap()
    ident_t = nc.dram_tensor("ident", [O, O], F32, kind="ExternalInput").ap()
    bias3_t = nc.dram_tensor("bias3", [3, O], F32, kind="ExternalInput").ap()
    out_t = nc.dram_tensor("out", [nb, O, hh, ww], FP16, kind="ExternalOutput").ap()

    with TileContext(nc) as tc, ExitStack() as stack:
        # ---------------- weight + bias sampling (one-time prologue) --------
        cp = stack.enter_context(tc.tile_pool(name="consts", bufs=1))
        wl = cp.tile([O, I * D * D], BF16, name="wl", tag="wl")
        pbig = cp.tile([O, I * D * D], BF16, name="pbig", tag="pbig")
        wloc = cp.tile([O, I * D], F32, name="wloc_s", tag="wloc_s")
        epsw = cp.tile([O, I * D], BF16, name="epsw_s", tag="epsw_s")
        ident = cp.tile([O, O], F32, name="ident_s", tag="ident_s")
        b3 = cp.tile([O, 3], F32, name="b3", tag="b3")
        spb = cp.tile([O, I * D], BF16, name="spb", tag="spb")
        sp = cp.tile([O, I * D], F32, name="sp", tag="sp")
        wsamp = cp.tile([O, I * D], F32, name="wsamp", tag="wsamp")
        bias = cp.tile([128, 1], F32, name="bias", tag="bias")
        # 9 transposed bf16 tap tiles side by side, duplicated on both
        # partition halves: wts[64r:64r+64, 64t:64t+64] = T_t with
        # T_t[i, o] = wsamp[o, i*9 + t]
        wts = cp.tile([128, D * O], BF16, name="wts", tag="wts")
        sp_b = cp.tile([O, 1], F32, name="sp_b", tag="sp_b")

        b3p = cp.tile([3, O], F32, name="b3p", tag="b3p")
        # split the wl load across both HWDGE queues so it lands sooner (the
        # whole sampling chain waits on it).
        csz = I * D * D // 2
        for qi, q in enumerate((nc.sync, nc.scalar)):
            q.dma_start(
                wl[:, qi * csz : (qi + 1) * csz],
                wl_t[:, qi * csz : (qi + 1) * csz],
            )
        nc.scalar.dma_start(b3p[:], bias3_t[:])
        nc.scalar.dma_start(ident[:], ident_t[:])
        nc.scalar.dma_start(wloc[:], wloc_t[:])
        nc.scalar.dma_start(epsw[:], epsw_t[:])

        # PE warm-up feed: zero tiles via GpSimd (idle queue, no input deps)
        # so the warm-up matmuls can start within ~1us of kernel entry.
        identr = cp.tile([O, O], F32R, name="identr", tag="identr")
        junk = cp.tile([O, 256], F32R, name="junk", tag="junk")
        with tc.high_priority():
            nc.gpsimd.memset(identr[:].bitcast(F32), 0.0)
            nc.gpsimd.memset(junk[:].bitcast(F32), 0.0)

        # PE warm-up: the HAM clock gate needs ~3.4us of sustained matmul
        # activity to lift the PE from 1.2 to 2.4 GHz, and re-throttles after
        # ~3.4us idle. One long accumulation group (no inter-matmul
        # semaphores) bridges the PE from kernel entry to the transposes.
        with tc.tile_pool(name="wp", bufs=1, space="PSUM") as wp:
            warm = wp.tile([O, 256], F32, name="warm")
            bp_ps = wp.tile([O, 3], F32, name="bp_ps")
            n_warm = 56
            for k in range(n_warm):
                nc.tensor.matmul(
                    warm[:], identr[:], junk[:],
                    start=(k == 0), stop=(k == n_warm - 1),
                    skip_group_check=True,
                )
                if k == 10:
                    # bias3 arrives as [3, 64]; transpose to [64, 3] on the PE
                    # (a partition-major DMA of 64x3 elements costs ~17us in
                    # descriptors). Placed inside the warm-up group so the
                    # dependent softplus chain is not blocked until the whole
                    # warm-up finishes.
                    with tc.high_priority():
                        nc.tensor.matmul(
                            bp_ps[:], b3p[:], ident[0:3, 0:3],
                            start=True, stop=True, skip_group_check=True,
                        )
                        nc.vector.tensor_copy(b3[:], bp_ps[:])

        # softplus of the per-(o,i) diagonals: wl free layout is (i, d=a*9+b);
        # diagonal entries sit at d = 10*a  ->  sp layout (i, a).
        # ACT order Exp,Exp,Ln,Ln avoids activation-table reload thrash
        # (each ACT_TABLE_LOAD costs ~1.3us). softplus(x) = ln(exp(x) + 1):
        # there is no Softplus LUT in this toolchain.
        def diag_ap():
            return bass.AP(
                tensor=wl[:].tensor,
                offset=wl[:].offset,
                ap=[list(p) for p in wl[:].ap[:1]] + [[D * D, I], [D + 1, D]],
            )

        spb3 = spb[:].rearrange("o (i a) -> o i a", i=I)
        with tc.high_priority():
            nc.scalar.activation(sp_b[:], b3[:, 1:2], AF.Exp)
            nc.scalar.activation(spb3, diag_ap(), AF.Exp)
            nc.scalar.activation(spb[:], spb[:], AF.Ln, bias=1.0)
            nc.scalar.activation(sp_b[:], sp_b[:], AF.Ln, bias=1.0)

        # scatter softplus(diag) back into the diagonal slots of the
        # (host-pre-masked) wl, then the whole L @ eps contraction is one
        # multiply + one segmented reduce (bf16 inputs: 2x DVE throughput;
        # the reduce accumulates into f32):
        #   P[o,i,a,b] = L[o,i,a,b] * eps[o,i,b];  wsamp = wloc + sum_b P
        nc.vector.tensor_copy(diag_ap(), spb3)
        wl4 = wl[:].rearrange("o (i a b) -> o i a b", i=I, a=D)
        p4 = pbig[:].rearrange("o (i a b) -> o i a b", i=I, a=D)
        eps_b4 = bass.AP(
            tensor=epsw[:].tensor,
            offset=epsw[:].offset,
            ap=[list(p) for p in epsw[:].ap[:1]] + [[D, I], [0, D], [1, D]],
        )
        nc.vector.tensor_tensor(p4, wl4, eps_b4, mybir.AluOpType.mult)
        nc.vector.tensor_reduce(
            sp[:].rearrange("o (i a) -> o i a", i=I),
            p4,
            axis=mybir.AxisListType.X,
            op=mybir.AluOpType.add,
        )
        nc.vector.tensor_add(wsamp[:], wloc[:], sp[:])

        # bias = bias_loc + eps_b * softplus(bias_ro) (after the sampling DVE
        # ops: nothing reads bias until the first PSUM evacuation)
        nc.vector.tensor_mul(sp_b[:], sp_b[:], b3[:, 2:3])
        nc.vector.tensor_add(bias[0:O, :], b3[:, 0:1], sp_b[:])
        nc.scalar.dma_start(bias[O:128, :], bias[0:O, :])

        # transpose the 9 taps on the PE (packed 5 + 4 into two PSUM banks),
        # then cast f32 -> bf16 into the wts tile, taps side by side.
        with tc.tile_pool(name="pt", bufs=1, space="PSUM") as ptp:
            ptA = ptp.tile([O, 5 * O], F32, name="ptA")
            ptB = ptp.tile([O, 4 * O], F32, name="ptB")
            for a in range(D):
                w_a = bass.AP(
                    tensor=wsamp[:].tensor,
                    offset=wsamp[:].offset + a,
                    ap=[list(p) for p in wsamp[:].ap[:1]] + [[D, I]],
                )
                dst_pt = ptA if a < 5 else ptB
                c = a if a < 5 else a - 5
                nc.tensor.matmul(
                    dst_pt[:, c * O : (c + 1) * O],
                    w_a,
                    ident[:],
                    is_transpose=True,
                    start=(c == 0),
                    stop=(c == (4 if a < 5 else 3)),
                    skip_group_check=True,
                )
            nc.vector.tensor_copy(wts[0:O, 0 : 5 * O], ptA[:])
            nc.vector.tensor_copy(wts[0:O, 5 * O : D * O], ptB[:])
        # duplicate the taps onto partitions 64-127 (for the quadrant streams
        # whose lhsT/rhs base partition is 64). Two DMAs so the first conv
        # matmuls only wait on the first chunk (taps 0-4 incl. TAP_ORDER[0]=4).
        for c0, c1 in ((0, 5 * O), (5 * O, D * O)):
            nc.scalar.dma_start(wts[O:128, c0:c1], wts[0:O, c0:c1])

        # ---------------- convolution ---------------------------------------
        # Quadrant streams (lhsT/rhs base partition r, psum base partition c):
        #   (0,0): img A -> psum[0:64]    (64,64): img B -> psum[64:128]
        #   (64,0): img B -> psum[0:64]   (0,64):  img A -> psum[64:128]
        # Even psum tiles j use (0,0)+(64,64) -> [A;B];
        # odd tiles use (64,0)+(0,64) -> [B;A]. Store DMAs unscramble.
        xp = stack.enter_context(tc.tile_pool(name="xstrip", bufs=x_bufs))
        op = stack.enter_context(tc.tile_pool(name="ostrip", bufs=o_bufs))
        pp = stack.enter_context(tc.tile_pool(name="acc", bufs=8, space="PSUM"))

        xrows = rstrip + 2
        for pair in range(nb // 2):
            n0 = 2 * pair
            strips = [(s * rstrip, rstrip) for s in range(nstrips)]
            if pair == nb // 2 - 1:
                # Taper: split the final strip so the kernel does not end on
                # one full-size evac + store burst.
                h_last = strips.pop()[0]
                strips += [
                    (h_last, rstrip // 2),
                    (h_last + rstrip // 2, rstrip // 4),
                    (h_last + 3 * rstrip // 4, rstrip // 4),
                ]
            for h0, rs in strips:
                # load x rows [h0-1, h0+rs], clipped to the image.
                # tile row t holds x row (h0 - 1 + t); unloaded edge rows
                # are never referenced by the matmul APs below.
                r_lo = max(h0 - 1, 0)
                r_hi = min(h0 + rs + 1, hh)
                dst_r0 = r_lo - (h0 - 1)
                xs = xp.tile([128, xrows, ww], BF16, name="xs")
                src = x_t[n0 : n0 + 2, :, r_lo:r_hi, :].rearrange(
                    "n i h w -> (n i) h w"
                )
                nc.sync.dma_start(xs[:, dst_r0 : dst_r0 + (r_hi - r_lo), :], src)

                os_ = op.tile([128, rstrip, ww], FP16, name="os_")
                # groups of 4 psum tiles (8 output rows); taps iterate outside
                # the tiles so same-quadrant matmuls with the same weights are
                # adjacent in issue order.
                for g in range(rs // 8):
                    accs = [pp.tile([128, 2, ww], F32, name="acc") for _ in range(4)]
                    for ti, tap in enumerate(TAP_ORDER):
                        dy, dx = tap // 3 - 1, tap % 3 - 1
                        if dx == -1:
                            xc, oc, ncol = 0, 1, ww - 1
                        elif dx == 1:
                            xc, oc, ncol = 1, 0, ww - 1
                        else:
                            xc, oc, ncol = 0, 0, ww
                        # (tile j within group, rhs/lhsT base, psum base);
                        # runs of equal (base, tap) keep the weights loaded.
                        for jj, p, q in (
                            (0, 0, 0), (2, 0, 0), (1, 0, 64), (3, 0, 64),
                            (0, 64, 64), (2, 64, 64), (1, 64, 0), (3, 64, 0),
                        ):
                            j = 4 * g + jj
                            R = h0 + 2 * j
                            xr_lo = max(R + dy, 0)
                            xr_hi = min(R + 1 + dy, hh - 1)
                            nr = xr_hi - xr_lo + 1
                            orow = xr_lo - dy - R
                            trow = xr_lo - (h0 - 1)
                            acc = accs[jj]
                            nc.tensor.matmul(
                                acc[q : q + 64, orow : orow + nr, oc : oc + ncol],
                                wts[p : p + 64, tap * O : (tap + 1) * O],
                                xs[p : p + 64, trow : trow + nr, xc : xc + ncol],
                                start=(ti == 0),
                                stop=(ti == D - 1),
                                skip_group_check=True,
                            )
                    for jj in range(4):
                        j = 4 * g + jj
                        # alternate evacuation between ScalarE and VectorE so
                        # PSUM banks free up twice as fast
                        if jj % 2 == 0:
                            nc.scalar.activation(
                                os_[:, 2 * j : 2 * j + 2, :],
                                accs[jj][:],
                                AF.Identity,
                                bias=bias[:, 0:1],
                            )
                        else:
                            nc.vector.tensor_scalar_add(
                                os_[:, 2 * j : 2 * j + 2, :],
                                accs[jj][:],
                                bias[:, 0:1],
                            )
                # store with the odd-tile half swap undone: partitions 0-63
                # hold img A rows (4m, 4m+1) and img B rows (4m+2, 4m+3);
                # partitions 64-127 the complement; split across both HWDGE
                # queues.
                nm = rs // 4
                for img, part0, rbase, q in (
                    (0, 0, 0, nc.sync),
                    (1, 64, 0, nc.sync),
                    (0, 64, 2, nc.sync),
                    (1, 0, 2, nc.sync),
                ):
                    src_h = os_[part0 : part0 + 64]
                    src = bass.AP(
                        tensor=src_h.tensor,
                        offset=src_h.offset + rbase * ww,
                        ap=[list(p) for p in src_h.ap[:1]]
                        + [[4 * ww, nm], [ww, 2], [1, ww]],
                    )
                    dst = bass.AP(
                        tensor=out_t.tensor,
                        offset=out_t.offset
                        + ((n0 + img) * O * hh + h0 + rbase) * ww,
                        ap=[[hh * ww, O], [4 * ww, nm], [ww, 2], [1, ww]],
                    )
                    q.dma_start(dst, src)

    nc.compile()
    return nc


_CACHED_NC = None


def _host_inputs(x_shard, weight_loc, weight_L, bias_loc, bias_ro, eps_w, eps_b):
    import ml_dtypes

    # strict-lower mask applied on the host (structural constant of the
    # architecture); the raw diagonal stays in place for the device-side
    # softplus. Upper triangle is zeroed so the device-side L @ eps
    # contraction needs no masking.
    wl = np.asarray(weight_L, np.float32).reshape(O, I, D, D)
    tril = np.tril(np.ones((D, D), np.float32))
    wl = (wl * tril[None, None]).astype(ml_dtypes.bfloat16)

    return {
        "x": np.ascontiguousarray(
            np.asarray(x_shard, np.float32).astype(ml_dtypes.bfloat16)
        ),
        "wL": np.ascontiguousarray(wl.reshape(O, I * D * D)),
        "wloc": np.ascontiguousarray(weight_loc.reshape(O, I * D), np.float32),
        "epsw": np.ascontiguousarray(
            np.asarray(eps_w, np.float32).reshape(O, I * D).astype(ml_dtypes.bfloat16)
        ),
        "ident": np.eye(O, dtype=np.float32),
        "bias3": np.ascontiguousarray(
            np.stack([bias_loc, bias_ro, eps_b]).astype(np.float32)
        ),
    }


def kernel(x, weight_loc, weight_L, bias_loc, bias_ro, eps_w, eps_b):
    global _CACHED_NC
    from concourse.bass_utils import run_bass_kernel_spmd

    x = np.asarray(x, np.float32)
    nb = x.shape[0] // N_CORES
    if _CACHED_NC is None:
        _CACHED_NC = build_nc(nb=nb)
    nc = _CACHED_NC

    in_maps = [
        _host_inputs(
            x[c * nb : (c + 1) * nb],
            np.asarray(weight_loc),
            np.asarray(weight_L),
            np.asarray(bias_loc),
            np.asarray(bias_ro),
            np.asarray(eps_w),
            np.asarray(eps_b),
        )
        for c in range(N_CORES)
    ]
    res = run_bass_kernel_spmd(nc, in_maps, list(range(N_CORES)))
    return np.concatenate(
        [res.results[c]["out"].astype(np.float32) for c in range(N_CORES)], axis=0
    )
